# revision 12
# baseline (speedup 1.0000x reference)
"""Causal multi-head attention (B=2, T=2048, C=1024, H=16) on 8 trn2 NeuronCores.

Sharding: core = (b, g): b = core // 4 (batch), g = core % 4 (head group of 4
heads).  Each core:
  xp^T = (x[b] + pe)^T  (pe is a constant; folded on host into the f16
  layout/cast pass)                                     [C, T]
  qk^T = w_qk_local^T.T @ xp^T                          [512, T]  (q,k of 4 heads, transposed)
  v    = xp^T.T @ w_v_local^T                           [T, 256]  (natural layout, + ones col)
  per head h, per 512-wide query block: scores^T = k_h^T.T @ q_h^T (causal,
  lower j-tiles only), p^T = exp(scores^T / 8 - 3) (masked diag),
  y^T = [v|1]^T.T @ p^T accumulated over j-tiles -> row 64 of y^T is the
  softmax denominator (the -3 bias cancels in the ratio; it keeps f16 safe).
  y_cat^T[c_local, t] = y^T * recip(denom); out_partial = y_cat^T.T @ w_proj_local^T
Host sums the 4 partial outputs per batch (unshard of the row-sharded proj).

Perf structure: the two heads of a pair use lhsT/rhs at base partitions 0/64,
so their K=64 score matmuls land in different PE row-groups and co-execute
(tile_position row tiling).  qkv-projection / proj m-tile units are woven
between score->exp->AV groups as PE fillers so the in-order PE queue has work
during exp latency.  Softmax recip runs on DVE (reciprocal_approx_fast); each
pair's normalize chain is deferred to the next weave point so it never sits
ahead of the next group's masks in the in-order DVE queue.  The final proj
block starts its kk=0 matmuls during the last norm chain (PSUM borrowed from
the then-free score tiles).

All matmuls run in float16 (full-rate PE mode, fp32 PSUM accumulation).
"""

import numpy as np

B, T, C, H = 2, 2048, 1024, 16
NCORES = 8
GROUPS = 4            # head-groups across cores (tensor parallel)
HL = H // GROUPS      # heads per core = 4
D = C // H            # 64
CL = HL * D           # 256 local channels
KC = C // 128         # 8 contraction tiles over C
JG = 2                # j-tiles per scores psum tile (exp batch)
N_WARM = 56           # PE warmup matmuls (HAM un-throttle during initial DMA)

_PROG_CACHE = {}


def _build_program(t_len=T):
    from contextlib import ExitStack

    import concourse.tile as tile
    from concourse import bacc, mybir
    from concourse.masks import make_upper_triangular

    f32 = mybir.dt.float32
    f16 = mybir.dt.float16

    nt = t_len // 512     # 512-wide t chunks
    mt_n = t_len // 128   # 128-wide t tiles

    nc = bacc.Bacc("TRN2", target_bir_lowering=False, debug=False,
                   num_devices=NCORES)

    # chunk-major, partition-tiled host layouts -> fully contiguous DMAs
    x_r = nc.dram_tensor("x_t", [t_len // 512, 128, KC, 512], f16,
                         kind="ExternalInput").ap()
    wqk_r = nc.dram_tensor("w_qk_t", [128, KC, 2 * CL], f16,
                           kind="ExternalInput").ap()
    wv_r = nc.dram_tensor("w_v_t", [128, KC, CL], f16,
                          kind="ExternalInput").ap()
    wproj_r = nc.dram_tensor("w_proj_t", [128, CL // 128, C], f16,
                             kind="ExternalInput").ap()
    out = nc.dram_tensor("out", [t_len, C], f16, kind="ExternalOutput").ap()

    with tile.TileContext(nc) as tc:
        with ExitStack() as ctx:
            const_p = ctx.enter_context(tc.tile_pool(name="const", bufs=1))
            w_p = ctx.enter_context(tc.tile_pool(name="weights", bufs=1))
            act_p = ctx.enter_context(tc.tile_pool(name="acts", bufs=1))
            work_p = ctx.enter_context(tc.tile_pool(name="work", bufs=2))
            ps_p = ctx.enter_context(tc.tile_pool(name="ps", bufs=2, space="PSUM"))

            # PE warmup first: N=128 dummy matmuls while the first DMAs land,
            # so the HAM clock gate is at 8/8 when real matmuls start.  wrm is
            # memset on gpsimd, which boots ~3us before the vector engine, so
            # warmups start at ~0.5us.  Results are never read; every real
            # matmul group begins with start=True which overwrites.
            # Rotate column slices so the WAW reuse distance is 4, not 2.
            wrm = const_p.tile([128, 128], f16, tag="wrm")
            nc.gpsimd.memset(wrm[:], 0.0)
            for i in range(N_WARM):
                wps = ps_p.tile([128, 512], f32, tag="mmps", name="wps")
                sl = (i % 4) * 128
                nc.tensor.matmul(wps[:, sl:sl + 128], lhsT=wrm[:], rhs=wrm[:],
                                 start=True, stop=True)

            # causal mask for diagonal score tiles (keep j <= i)
            mask_ut = const_p.tile([128, 128], f16, tag="mask")
            make_upper_triangular(nc, mask_ut[:], val=1.0, diag=True)
            ones_f32 = const_p.tile([128, HL], f32, tag="ones")
            nc.vector.memset(ones_f32[:], 1.0)
            expbias = const_p.tile([128, 1], f32, tag="expbias")
            nc.vector.memset(expbias[:], -3.0)

            wqk_sb = w_p.tile([128, KC, 2 * CL], f16, tag="wqk")
            wv_sb = w_p.tile([128, KC, CL], f16, tag="wv")
            wproj_sb = w_p.tile([128, CL // 128, C], f16, tag="wpj")

            # ---- persistent activations ----
            # qk^T rows: m0 = q heads 0,1; m1 = q heads 2,3; m2 = k h0,1; m3 = k h2,3
            qk_sb = [act_p.tile([128, t_len], f16, tag=f"qk{m}", name="qk")
                     for m in range(4)]
            # v tiles, per 128-t tile: 4 heads x (64 v cols + ones col)
            v_sb = [act_p.tile([128, HL * (D + 1)], f16, tag=f"v{m}", name="v")
                    for m in range(mt_n)]
            for m in range(mt_n):
                ones_col = v_sb[m].rearrange("p (h e) -> p h e", e=D + 1)[:, :, D:D + 1]
                nc.vector.tensor_copy(ones_col,
                                      ones_f32.rearrange("p (h o) -> p h o", o=1))
            # y_cat^T [256, T] as 2 tiles of 128 partitions
            ycat_sb = [act_p.tile([128, t_len], f16, tag=f"ycat{k}", name="ycat")
                       for k in range(CL // 128)]

            xs_of = {}

            def emit_chunk_dma(n):
                """One 1MiB DMA on the sync ring; x already has pe folded in
                host-side.  The scalar queue stays free of DMA issues."""
                x_s = work_p.tile([128, KC, 512], f16, tag="x", bufs=2,
                                  name="x_s")
                nc.sync.dma_start(out=x_s[:], in_=x_r[n])
                xs_of[n] = x_s

            def qk_unit(n, i):
                def go():
                    xs = xs_of[n]
                    ps = ps_p.tile([128, 512], f32, tag="mmps", name="qk_ps")
                    for ck in range(KC):
                        nc.tensor.matmul(
                            ps[:],
                            lhsT=wqk_sb[:, ck, i * 128:(i + 1) * 128],
                            rhs=xs[:, ck, :],
                            start=(ck == 0), stop=(ck == KC - 1))
                    nc.vector.tensor_copy(
                        qk_sb[i][:, n * 512:(n + 1) * 512], ps[:])
                return go

            def v_unit(n, i):
                def go():
                    xs = xs_of[n]
                    psv = ps_p.tile([128, CL], f32, tag="mmps", name="v_ps")
                    for ck in range(KC):
                        nc.tensor.matmul(
                            psv[:],
                            lhsT=xs[:, ck, i * 128:(i + 1) * 128],
                            rhs=wv_sb[:, ck, :],
                            start=(ck == 0), stop=(ck == KC - 1))
                    mt = 4 * n + i
                    nc.vector.tensor_copy(
                        v_sb[mt].rearrange("p (h e) -> p h e",
                                           e=D + 1)[:, :, 0:D],
                        psv.rearrange("p (h e) -> p h e", e=D))
                return go

            def chunk_units(n):
                return ([qk_unit(n, i) for i in range(4)]
                        + [v_unit(n, i) for i in range(4)])

            def proj_half(c, i, n2, fast_tail=False):
                """Half proj m-tile: out[512c+128i : +128, 512 n2 : +512]."""
                def go():
                    mt = 4 * c + i
                    ps = ps_p.tile([128, 512], f32, tag="mmps", name="proj_ps")
                    for kk in range(CL // 128):
                        nc.tensor.matmul(
                            ps[:],
                            lhsT=ycat_sb[kk][:, mt * 128:(mt + 1) * 128],
                            rhs=wproj_sb[:, kk, n2 * 512:(n2 + 1) * 512],
                            start=(kk == 0), stop=(kk == CL // 128 - 1))
                    osb = work_p.tile([128, 512], f16, tag="osb", bufs=4,
                                      name="osb")
                    if fast_tail:
                        nc.scalar.copy(osb[:], ps[:])
                    else:
                        nc.vector.tensor_copy(osb[:], ps[:])
                    ring = nc.sync if (mt + n2) % 2 == 0 else nc.gpsimd
                    ring.dma_start(
                        out=out[mt * 128:(mt + 1) * 128,
                                n2 * 512:(n2 + 1) * 512],
                        in_=osb[:])
                return go

            def proj_units(c):
                return [proj_half(c, i, n2) for i in range(4) for n2 in range(2)]

            pending_norm = []   # deferred per-pair normalize chains

            def norm_chain(c, hp, yps_of):
                """DVE/GPSIMD normalize for a head pair; emitted at a later
                weave point so it never sits ahead of critical masks in the
                in-order DVE queue.  copies+recips first, then broadcasts,
                then muls, so the gpsimd broadcast latency is pipelined."""
                def go():
                    rcs, rbs = {}, {}
                    for h in (hp, hp + 1):
                        dsb = work_p.tile([1, 512], f32, tag="dsb", bufs=4,
                                          name="dsb")
                        # stage via a tracked copy: custom-DVE PSUM reads
                        # bypass tile's dependency tracking
                        nc.vector.tensor_copy(dsb[:], yps_of[h][64:65, :])
                        rc = work_p.tile([1, 512], f32, tag="rc", bufs=4,
                                         name="rc")
                        nc.vector.reciprocal_approx_fast(rc[:], dsb[:])
                        rch = work_p.tile([1, 512], f16, tag="rch", bufs=4,
                                          name="rch")
                        nc.vector.tensor_copy(rch[:], rc[:])
                        rcs[h] = rch
                    for h in (hp, hp + 1):
                        # f16 broadcast: halves the gpsimd broadcast time
                        rb = work_p.tile([64, 512], f16, tag="rb", bufs=4,
                                         name="rb")
                        nc.gpsimd.partition_broadcast(rb[:], rcs[h][:])
                        rbs[h] = rb
                    for h in (hp, hp + 1):
                        hb = (h % 2) * 64
                        nc.vector.tensor_mul(
                            ycat_sb[h // 2][hb:hb + 64,
                                            c * 512:(c + 1) * 512],
                            yps_of[h][0:64, :], rbs[h][:])
                return go

            def emit_attn_block(c, fillers=()):
                """Attention for query block i in [512c, 512c+512), all heads.
                Head pairs (0,1)/(2,3) sit at partition bases 0/64 of the same
                qk tiles, so their K=64 score matmuls are emitted interleaved
                by j-tile and co-execute in different PE row-groups."""
                fillers = list(fillers)
                njt = 4 * c + 4
                for hp in (0, 2):
                    yps_of = {}
                    for h in (hp, hp + 1):
                        yps_of[h] = ps_p.tile([65, 512], f32, tag="yps",
                                              name="yps")
                    for g0 in range(0, njt, JG):
                        jts = range(g0, min(g0 + JG, njt))
                        sps_of, esb_of, offs = {}, {}, {}
                        for h in (hp, hp + 1):
                            sps_of[h] = ps_p.tile([128, JG * 512], f32,
                                                  tag="sps", name="sps")
                            esb_of[h] = work_p.tile([128, JG * 512], f16,
                                                    tag="esb", bufs=4,
                                                    name="esb")
                        for jt in jts:
                            offs[jt] = max(0, (jt - 4 * c)) * 128
                        # scores: interleave heads per j-tile -> row-group
                        # concurrency (h at base 0 runs with h+1 at base 64)
                        for jt in jts:
                            off = offs[jt]
                            ls = (jt - g0) * 512 + off
                            width = 512 - off
                            for h in (hp, hp + 1):
                                hb = (h % 2) * 64
                                nc.tensor.matmul(
                                    sps_of[h][:, ls:ls + width],
                                    lhsT=qk_sb[2 + h // 2][
                                        hb:hb + 64, jt * 128:(jt + 1) * 128],
                                    rhs=qk_sb[h // 2][
                                        hb:hb + 64,
                                        c * 512 + off:(c + 1) * 512],
                                    start=True, stop=True)
                        # exp; per-j-tile on diagonal groups (no dead gap)
                        for h in (hp, hp + 1):
                            esb = esb_of[h]
                            if offs[jts[-1]] > 0:
                                for jt in jts:
                                    ls = (jt - g0) * 512 + offs[jt]
                                    le = (jt - g0) * 512 + 512
                                    nc.scalar.activation(
                                        esb[:, ls:le], sps_of[h][:, ls:le],
                                        mybir.ActivationFunctionType.Exp,
                                        scale=0.125, bias=expbias[:])
                            else:
                                le = (jts[-1] - g0) * 512 + 512
                                nc.scalar.activation(
                                    esb[:, 0:le], sps_of[h][:, 0:le],
                                    mybir.ActivationFunctionType.Exp,
                                    scale=0.125, bias=expbias[:])
                            for jt in jts:
                                if jt >= 4 * c:  # diagonal tile: causal mask
                                    ls = (jt - g0) * 512 + offs[jt]
                                    nc.vector.tensor_mul(
                                        esb[:, ls:ls + 128],
                                        esb[:, ls:ls + 128], mask_ut[:])
                        if pending_norm:
                            pending_norm.pop(0)()
                        if fillers:
                            fillers.pop(0)()
                        for h in (hp, hp + 1):
                            esb = esb_of[h]
                            for jt in jts:
                                off = offs[jt]
                                ls = (jt - g0) * 512 + off
                                nc.tensor.matmul(
                                    yps_of[h][:, off:512],
                                    lhsT=v_sb[jt][:, h * (D + 1):
                                                  (h + 1) * (D + 1)],
                                    rhs=esb[:, ls:ls + (512 - off)],
                                    start=(jt == 0), stop=(jt == njt - 1))
                    pending_norm.append(norm_chain(c, hp, yps_of))
                for f in fillers:
                    f()
                del fillers[:]

            # attn(c) needs qkv chunks <= c; proj(c) needs attn(c).  Chunk
            # n+1's DMA is emitted before attn(n).  Weave plan (one PE filler
            # + one pending norm chain per score/exp/AV group):
            #   attn(0) <- qkv(1)            (8 units / 4 groups)
            #   attn(1) <- qkv(2)            (8 / 8)
            #   attn(2) <- qk(3) + proj(0)   (4+8 / 12)
            #   attn(3) <- v(3) + proj(1) + proj(2)  (4+8+8 / 16, rest at tail)
            #   tail: last norm chain overlapped with proj(3) kk=0 matmuls
            emit_chunk_dma(0)
            # weights after chunk-0 x: halves on the two free rings; wproj is
            # deferred (first needed by proj(0) fillers inside attn(2))
            nc.sync.dma_start(out=wqk_sb[:, 0:KC // 2, :],
                              in_=wqk_r[:, 0:KC // 2, :])
            nc.gpsimd.dma_start(out=wqk_sb[:, KC // 2:KC, :],
                                in_=wqk_r[:, KC // 2:KC, :])
            nc.gpsimd.dma_start(out=wv_sb[:], in_=wv_r[:])
            for i in range(4):      # qk units first (need only wqk)
                qk_unit(0, i)()
            for i in range(4):      # v units after (wv arrives later)
                v_unit(0, i)()
            emit_chunk_dma(1)
            nc.gpsimd.dma_start(out=wproj_sb[:], in_=wproj_r[:])
            emit_attn_block(0, chunk_units(1))
            emit_chunk_dma(2)
            emit_attn_block(1, chunk_units(2))
            emit_chunk_dma(3)
            emit_attn_block(2, [qk_unit(3, i) for i in range(4)]
                            + proj_units(0))
            emit_attn_block(3, [v_unit(3, i) for i in range(4)]
                            + proj_units(1) + proj_units(2))

            # ---- tail: overlap the last pair's norm chain with proj(3) ----
            # ycat_sb[0] (heads 0,1) is complete; ycat_sb[1] needs the last
            # chain.  kk=0 matmuls run during the chain in PSUM borrowed from
            # the now-free score tiles; kk=1 lands once ycat_sb[1] is ready.
            last_chain = pending_norm.pop(0)
            assert not pending_norm
            tail_ps = []
            for _ in range(2):
                sps = ps_p.tile([128, JG * 512], f32, tag="sps", name="sps")
                tail_ps.append(sps[:, 0:512])
                tail_ps.append(sps[:, 512:1024])
            for _ in range(2):
                mps = ps_p.tile([128, 512], f32, tag="mmps", name="proj_ps")
                tail_ps.append(mps[:])

            def tail_mm(ps, mt, n2, kk):
                nc.tensor.matmul(
                    ps,
                    lhsT=ycat_sb[kk][:, mt * 128:(mt + 1) * 128],
                    rhs=wproj_sb[:, kk, n2 * 512:(n2 + 1) * 512],
                    start=(kk == 0), stop=(kk == CL // 128 - 1))

            slots = [(0, 0), (0, 1), (1, 0), (1, 1), (2, 0), (2, 1)]
            for s, (i, n2) in enumerate(slots):
                tail_mm(tail_ps[s], 12 + i, n2, 0)
            last_chain()

            def tail_out(ps, i, n2, on_scalar):
                osb = work_p.tile([128, 512], f16, tag="osb", bufs=4,
                                  name="osb")
                if on_scalar:
                    nc.scalar.copy(osb[:], ps)
                else:
                    nc.vector.tensor_copy(osb[:], ps)
                ring = nc.sync if on_scalar else nc.gpsimd
                ring.dma_start(out=out[(12 + i) * 128:(13 + i) * 128,
                                       n2 * 512:(n2 + 1) * 512], in_=osb[:])

            for s, (i, n2) in enumerate(slots):
                tail_mm(tail_ps[s], 12 + i, n2, 1)
                tail_out(tail_ps[s], i, n2, on_scalar=(s % 2 == 0))
            # mt 15 on a third borrowed score tile (WAR on slots 0/1, long
            # done); both halves' matmuls back-to-back, copies in parallel
            # on ACT and DVE
            sps3 = ps_p.tile([128, JG * 512], f32, tag="sps", name="sps")
            for n2 in range(2):
                for kk in range(CL // 128):
                    tail_mm(sps3[:, n2 * 512:(n2 + 1) * 512], 15, n2, kk)
            for n2 in range(2):
                tail_out(sps3[:, n2 * 512:(n2 + 1) * 512], 3, n2,
                         on_scalar=(n2 == 0))

    nc.compile()
    return nc


def _shard_inputs(x, w_qkv, w_proj, pe, t_len=T):
    x = np.asarray(x, dtype=np.float32)
    pe = np.asarray(pe, dtype=np.float32)
    xp = (x + pe[None, :t_len, :]).astype(np.float16)   # fold the constant PE
    w_qkv = np.asarray(w_qkv, dtype=np.float32).astype(np.float16)
    w_proj = np.asarray(w_proj, dtype=np.float32).astype(np.float16)

    def chunk_major(a_t):      # [C, t] -> [nt, 128, KC, 512]
        return np.ascontiguousarray(
            a_t.reshape(KC, 128, t_len // 512, 512).transpose(2, 1, 0, 3))

    def part_tiled(w_t):       # [C_in, M] -> [128, C_in//128, M]
        return np.ascontiguousarray(
            w_t.reshape(-1, 128, w_t.shape[1]).transpose(1, 0, 2))

    x_ts = [chunk_major(xp[b, :t_len].T) for b in range(x.shape[0])]
    in_maps = []
    for core in range(NCORES):
        b, g = core // GROUPS, core % GROUPS
        rows_q = w_qkv[g * CL:(g + 1) * CL]
        rows_k = w_qkv[C + g * CL:C + (g + 1) * CL]
        rows_v = w_qkv[2 * C + g * CL:2 * C + (g + 1) * CL]
        in_maps.append({
            "x_t": x_ts[b],
            "w_qk_t": part_tiled(np.concatenate([rows_q, rows_k], axis=0).T.copy()),
            "w_v_t": part_tiled(rows_v.T.copy()),
            "w_proj_t": part_tiled(w_proj[:, g * CL:(g + 1) * CL].T.copy()),
        })
    return in_maps


_RUN_KWARGS = {}       # test-harness hook (e.g. trace=True); empty when graded
_LAST_RESULT = None


def kernel(x, w_qkv, w_proj, pe):
    global _LAST_RESULT
    from concourse import bass_utils

    if T not in _PROG_CACHE:
        _PROG_CACHE[T] = _build_program(T)
    nc = _PROG_CACHE[T]

    in_maps = _shard_inputs(x, w_qkv, w_proj, pe)
    res = bass_utils.run_bass_kernel_spmd(nc, in_maps, core_ids=list(range(NCORES)),
                                          **_RUN_KWARGS)
    _LAST_RESULT = res

    out = np.zeros((B, T, C), dtype=np.float32)
    for core in range(NCORES):
        out[core // GROUPS] += res.results[core]["out"].astype(np.float32)
    return out


# revision 20
# speedup vs baseline: 1.0408x; 1.0408x over previous
"""Causal multi-head attention (B=2, T=2048, C=1024, H=16) on 8 trn2 NeuronCores.

Sharding: core = (b, g): b = core // 4 (batch), g = core % 4 (head group of 4
heads).  Each core:
  xp^T = (x[b] + pe)^T  (pe is a constant; folded on host into the f16
  layout/cast pass)                                     [C, T]
  qk^T = w_qk_local^T.T @ xp^T                          [512, T]  (q,k of 4 heads, transposed)
  v    = xp^T.T @ w_v_local^T                           [T, 256]  (natural layout, + ones col)
  per head h, per 512-wide query block: scores^T = k_h^T.T @ q_h^T (causal,
  lower j-tiles only), p^T = exp(scores^T / 8 - 3) (masked diag),
  y^T = [v|1]^T.T @ p^T accumulated over j-tiles -> row 64 of y^T is the
  softmax denominator (the -3 bias cancels in the ratio; it keeps f16 safe).
  y_cat^T[c_local, t] = y^T * recip(denom); out_partial = y_cat^T.T @ w_proj_local^T
Host sums the 4 partial outputs per batch (unshard of the row-sharded proj).

Perf structure: the two heads of a pair use lhsT/rhs at base partitions 0/64,
so their K=64 score matmuls land in different PE row-groups and co-execute
(tile_position row tiling).  qkv-projection / proj m-tile units are woven
between score->exp->AV groups as PE fillers so the in-order PE queue has work
during exp latency.  Softmax recip runs on DVE (reciprocal_approx_fast); each
pair's normalize chain is deferred to the next weave point so it never sits
ahead of the next group's masks in the in-order DVE queue.  The final proj
block starts its kk=0 matmuls during the last norm chain (PSUM borrowed from
the then-free score tiles).

All matmuls run in float16 (full-rate PE mode, fp32 PSUM accumulation).
"""

import numpy as np

B, T, C, H = 2, 2048, 1024, 16
NCORES = 8
GROUPS = 4            # head-groups across cores (tensor parallel)
HL = H // GROUPS      # heads per core = 4
D = C // H            # 64
CL = HL * D           # 256 local channels
KC = C // 128         # 8 contraction tiles over C
JG = 2                # j-tiles per scores psum tile (exp batch)
N_WARM = 72           # PE warmup matmuls (HAM un-throttle during initial DMA)

_PROG_CACHE = {}


def _build_program(t_len=T):
    from contextlib import ExitStack

    import concourse.tile as tile
    from concourse import bacc, mybir
    from concourse.masks import make_upper_triangular

    f32 = mybir.dt.float32
    f16 = mybir.dt.float16

    nt = t_len // 512     # 512-wide t chunks
    mt_n = t_len // 128   # 128-wide t tiles

    nc = bacc.Bacc("TRN2", target_bir_lowering=False, debug=False,
                   num_devices=NCORES)

    # chunk-major, partition-tiled host layouts -> fully contiguous DMAs
    x_r = nc.dram_tensor("x_t", [t_len // 512, 128, KC, 512], f16,
                         kind="ExternalInput").ap()
    wqk_r = nc.dram_tensor("w_qk_t", [128, KC, 2 * CL], f16,
                           kind="ExternalInput").ap()
    wv_r = nc.dram_tensor("w_v_t", [128, KC, CL], f16,
                          kind="ExternalInput").ap()
    wproj_r = nc.dram_tensor("w_proj_t", [128, CL // 128, C], f16,
                             kind="ExternalInput").ap()
    out = nc.dram_tensor("out", [t_len, C], f16, kind="ExternalOutput").ap()

    with tile.TileContext(nc) as tc:
        with ExitStack() as ctx:
            const_p = ctx.enter_context(tc.tile_pool(name="const", bufs=1))
            w_p = ctx.enter_context(tc.tile_pool(name="weights", bufs=1))
            act_p = ctx.enter_context(tc.tile_pool(name="acts", bufs=1))
            work_p = ctx.enter_context(tc.tile_pool(name="work", bufs=2))
            ps_p = ctx.enter_context(tc.tile_pool(name="ps", bufs=2, space="PSUM"))

            # PE warmup first: N=128 dummy matmuls while the first DMAs land,
            # so the HAM clock gate is at 8/8 when real matmuls start.  wrm is
            # memset on gpsimd, which boots ~3us before the vector engine, so
            # warmups start at ~0.5us.  Results are never read; every real
            # matmul group begins with start=True which overwrites.
            # Rotate column slices so the WAW reuse distance is 4, not 2.
            wrm = const_p.tile([128, 128], f16, tag="wrm")
            nc.gpsimd.memset(wrm[:], 0.0)
            for i in range(N_WARM):
                wps = ps_p.tile([128, 512], f32, tag="mmps", name="wps")
                sl = (i % 4) * 128
                nc.tensor.matmul(wps[:, sl:sl + 128], lhsT=wrm[:], rhs=wrm[:],
                                 start=True, stop=True)

            # causal mask for diagonal score tiles (keep j <= i)
            mask_ut = const_p.tile([128, 128], f16, tag="mask")
            make_upper_triangular(nc, mask_ut[:], val=1.0, diag=True)
            ones_f32 = const_p.tile([128, HL], f32, tag="ones")
            nc.vector.memset(ones_f32[:], 1.0)
            expbias = const_p.tile([128, 1], f32, tag="expbias")
            nc.vector.memset(expbias[:], -3.0)

            wqk_sb = w_p.tile([128, KC, 2 * CL], f16, tag="wqk")
            wv_sb = w_p.tile([128, KC, CL], f16, tag="wv")
            wproj_sb = w_p.tile([128, CL // 128, C], f16, tag="wpj")

            # ---- persistent activations ----
            # qk^T rows: m0 = q heads 0,1; m1 = q heads 2,3; m2 = k h0,1; m3 = k h2,3
            qk_sb = [act_p.tile([128, t_len], f16, tag=f"qk{m}", name="qk")
                     for m in range(4)]
            # v tiles, per 128-t tile: 4 heads x (64 v cols + ones col)
            v_sb = [act_p.tile([128, HL * (D + 1)], f16, tag=f"v{m}", name="v")
                    for m in range(mt_n)]
            for m in range(mt_n):
                ones_col = v_sb[m].rearrange("p (h e) -> p h e", e=D + 1)[:, :, D:D + 1]
                nc.vector.tensor_copy(ones_col,
                                      ones_f32.rearrange("p (h o) -> p h o", o=1))
            # y_cat^T [256, T] as 2 tiles of 128 partitions
            ycat_sb = [act_p.tile([128, t_len], f16, tag=f"ycat{k}", name="ycat")
                       for k in range(CL // 128)]

            xs_of = {}

            def emit_chunk_dma(n):
                """One 1MiB DMA on the sync ring; x already has pe folded in
                host-side.  The scalar queue stays free of DMA issues."""
                x_s = work_p.tile([128, KC, 512], f16, tag="x", bufs=2,
                                  name="x_s")
                nc.sync.dma_start(out=x_s[:], in_=x_r[n])
                xs_of[n] = x_s

            def qk_unit(n, i):
                def go():
                    xs = xs_of[n]
                    ps = ps_p.tile([128, 512], f32, tag="mmps", name="qk_ps")
                    for ck in range(KC):
                        nc.tensor.matmul(
                            ps[:],
                            lhsT=wqk_sb[:, ck, i * 128:(i + 1) * 128],
                            rhs=xs[:, ck, :],
                            start=(ck == 0), stop=(ck == KC - 1))
                    nc.vector.tensor_copy(
                        qk_sb[i][:, n * 512:(n + 1) * 512], ps[:])
                return go

            def v_unit(n, i):
                def go():
                    xs = xs_of[n]
                    psv = ps_p.tile([128, CL], f32, tag="mmps", name="v_ps")
                    for ck in range(KC):
                        nc.tensor.matmul(
                            psv[:],
                            lhsT=xs[:, ck, i * 128:(i + 1) * 128],
                            rhs=wv_sb[:, ck, :],
                            start=(ck == 0), stop=(ck == KC - 1))
                    mt = 4 * n + i
                    nc.vector.tensor_copy(
                        v_sb[mt].rearrange("p (h e) -> p h e",
                                           e=D + 1)[:, :, 0:D],
                        psv.rearrange("p (h e) -> p h e", e=D))
                return go

            def chunk_units(n):
                return ([qk_unit(n, i) for i in range(4)]
                        + [v_unit(n, i) for i in range(4)])

            def proj_half(c, i, n2, fast_tail=False):
                """Half proj m-tile: out[512c+128i : +128, 512 n2 : +512]."""
                def go():
                    mt = 4 * c + i
                    ps = ps_p.tile([128, 512], f32, tag="mmps", name="proj_ps")
                    for kk in range(CL // 128):
                        nc.tensor.matmul(
                            ps[:],
                            lhsT=ycat_sb[kk][:, mt * 128:(mt + 1) * 128],
                            rhs=wproj_sb[:, kk, n2 * 512:(n2 + 1) * 512],
                            start=(kk == 0), stop=(kk == CL // 128 - 1))
                    osb = work_p.tile([128, 512], f16, tag="osb", bufs=4,
                                      name="osb")
                    if fast_tail:
                        nc.scalar.copy(osb[:], ps[:])
                    else:
                        nc.vector.tensor_copy(osb[:], ps[:])
                    ring = nc.sync if (mt + n2) % 2 == 0 else nc.gpsimd
                    ring.dma_start(
                        out=out[mt * 128:(mt + 1) * 128,
                                n2 * 512:(n2 + 1) * 512],
                        in_=osb[:])
                return go

            def proj_units(c):
                return [proj_half(c, i, n2) for i in range(4) for n2 in range(2)]

            pending_norm = []   # deferred per-pair normalize chains

            def norm_chain(c, hp, yps_of):
                """DVE/GPSIMD normalize for a head pair; emitted at a later
                weave point so it never sits ahead of critical masks in the
                in-order DVE queue.  copies+recips first, then broadcasts,
                then muls, so the gpsimd broadcast latency is pipelined."""
                def go():
                    # stage the full yps to SBUF FIRST: this is the last
                    # reader of the yps PSUM bank, so copying everything up
                    # front frees the bank for the next pair's AV ~2us
                    # earlier than letting recip/mul read PSUM directly.
                    # (Also keeps the custom-DVE recip off PSUM, whose reads
                    # bypass tile's dependency tracking.)
                    ysbs, rbs = {}, {}
                    for h in (hp, hp + 1):
                        ysb = work_p.tile([65, 512], f32, tag="ysb", bufs=4,
                                          name="ysb")
                        nc.vector.tensor_copy(ysb[:], yps_of[h][:])
                        ysbs[h] = ysb
                    for h in (hp, hp + 1):
                        # recip must read a base-partition-0 AP: the custom
                        # DVE op mishandles nonzero base partitions
                        dsb = work_p.tile([1, 512], f32, tag="dsb", bufs=4,
                                          name="dsb")
                        nc.vector.tensor_copy(dsb[:], ysbs[h][64:65, :])
                        rc = work_p.tile([1, 512], f32, tag="rc", bufs=4,
                                         name="rc")
                        nc.vector.reciprocal_approx_fast(rc[:], dsb[:])
                        rb = work_p.tile([64, 512], f32, tag="rb", bufs=4,
                                         name="rb")
                        nc.gpsimd.partition_broadcast(rb[:], rc[:])
                        rbs[h] = rb
                    for h in (hp, hp + 1):
                        hb = (h % 2) * 64
                        nc.vector.tensor_mul(
                            ycat_sb[h // 2][hb:hb + 64,
                                            c * 512:(c + 1) * 512],
                            ysbs[h][0:64, :], rbs[h][:])
                return go

            def emit_attn_block(c, fillers=(), fpp=1):
                """Attention for query block i in [512c, 512c+512), all heads.
                Head pairs (0,1)/(2,3) sit at partition bases 0/64 of the same
                qk tiles, so their K=64 score matmuls are emitted interleaved
                by j-tile and co-execute in different PE row-groups."""
                fillers = list(fillers)
                njt = 4 * c + 4
                for hp in (0, 2):
                    yps_of = {}
                    for h in (hp, hp + 1):
                        yps_of[h] = ps_p.tile([65, 512], f32, tag="yps",
                                              name="yps")
                    for g0 in range(0, njt, JG):
                        jts = range(g0, min(g0 + JG, njt))
                        sps_of, esb_of, offs = {}, {}, {}
                        for h in (hp, hp + 1):
                            sps_of[h] = ps_p.tile([128, JG * 512], f32,
                                                  tag="sps", name="sps")
                            esb_of[h] = work_p.tile([128, JG * 512], f16,
                                                    tag="esb", bufs=4,
                                                    name="esb")
                        for jt in jts:
                            offs[jt] = max(0, (jt - 4 * c)) * 128
                        # scores: interleave heads per j-tile -> row-group
                        # concurrency (h at base 0 runs with h+1 at base 64)
                        for jt in jts:
                            off = offs[jt]
                            ls = (jt - g0) * 512 + off
                            width = 512 - off
                            for h in (hp, hp + 1):
                                hb = (h % 2) * 64
                                nc.tensor.matmul(
                                    sps_of[h][:, ls:ls + width],
                                    lhsT=qk_sb[2 + h // 2][
                                        hb:hb + 64, jt * 128:(jt + 1) * 128],
                                    rhs=qk_sb[h // 2][
                                        hb:hb + 64,
                                        c * 512 + off:(c + 1) * 512],
                                    start=True, stop=True)
                        # exp; per-j-tile on diagonal groups (no dead gap)
                        for h in (hp, hp + 1):
                            esb = esb_of[h]
                            if offs[jts[-1]] > 0:
                                for jt in jts:
                                    ls = (jt - g0) * 512 + offs[jt]
                                    le = (jt - g0) * 512 + 512
                                    nc.scalar.activation(
                                        esb[:, ls:le], sps_of[h][:, ls:le],
                                        mybir.ActivationFunctionType.Exp,
                                        scale=0.125, bias=expbias[:])
                            else:
                                le = (jts[-1] - g0) * 512 + 512
                                nc.scalar.activation(
                                    esb[:, 0:le], sps_of[h][:, 0:le],
                                    mybir.ActivationFunctionType.Exp,
                                    scale=0.125, bias=expbias[:])
                            for jt in jts:
                                if jt >= 4 * c:  # diagonal tile: causal mask
                                    ls = (jt - g0) * 512 + offs[jt]
                                    nc.vector.tensor_mul(
                                        esb[:, ls:ls + 128],
                                        esb[:, ls:ls + 128], mask_ut[:])
                        if pending_norm:
                            pending_norm.pop(0)()
                        for _ in range(fpp):
                            if fillers:
                                fillers.pop(0)()
                        for h in (hp, hp + 1):
                            esb = esb_of[h]
                            for jt in jts:
                                off = offs[jt]
                                ls = (jt - g0) * 512 + off
                                nc.tensor.matmul(
                                    yps_of[h][:, off:512],
                                    lhsT=v_sb[jt][:, h * (D + 1):
                                                  (h + 1) * (D + 1)],
                                    rhs=esb[:, ls:ls + (512 - off)],
                                    start=(jt == 0), stop=(jt == njt - 1))
                    pending_norm.append(norm_chain(c, hp, yps_of))
                for f in fillers:
                    f()
                del fillers[:]

            # attn(c) needs qkv chunks <= c; proj(c) needs attn(c).  Chunk
            # n+1's DMA is emitted before attn(n).  Weave plan (one PE filler
            # + one pending norm chain per score/exp/AV group):
            #   attn(0) <- qkv(1)            (8 units / 4 groups)
            #   attn(1) <- qkv(2)            (8 / 8)
            #   attn(2) <- qk(3) + proj(0)   (4+8 / 12)
            #   attn(3) <- v(3) + proj(1) + proj(2)  (4+8+8 / 16, rest at tail)
            #   tail: last norm chain overlapped with proj(3) kk=0 matmuls
            emit_chunk_dma(0)
            # weights after chunk-0 x: halves on the two free rings; wproj is
            # deferred (first needed by proj(0) fillers inside attn(2))
            nc.sync.dma_start(out=wqk_sb[:, 0:KC // 2, :],
                              in_=wqk_r[:, 0:KC // 2, :])
            nc.gpsimd.dma_start(out=wqk_sb[:, KC // 2:KC, :],
                                in_=wqk_r[:, KC // 2:KC, :])
            nc.gpsimd.dma_start(out=wv_sb[:], in_=wv_r[:])
            for i in range(4):      # qk units first (need only wqk)
                qk_unit(0, i)()
            emit_chunk_dma(1)
            nc.gpsimd.dma_start(out=wproj_sb[:], in_=wproj_r[:])
            # v(0) units weave into attn(0) itself (2 fillers per point,
            # just-in-time for each group's AV) so attention starts as soon
            # as the chunk-0 qk tiles exist
            emit_attn_block(0, [v_unit(0, i) for i in range(4)]
                            + chunk_units(1), fpp=2)
            emit_chunk_dma(2)
            emit_attn_block(1, chunk_units(2))
            emit_chunk_dma(3)
            emit_attn_block(2, [qk_unit(3, i) for i in range(4)]
                            + proj_units(0))
            emit_attn_block(3, [v_unit(3, i) for i in range(4)]
                            + proj_units(1) + proj_units(2))

            # ---- tail: overlap the last pair's norm chain with proj(3) ----
            # ycat_sb[0] (heads 0,1) is complete; ycat_sb[1] needs the last
            # chain.  kk=0 matmuls for mt 12/13 run before the chain in PSUM
            # borrowed from one free score tile + the mm pool; the OTHER
            # score tile hosts warm-keeper dummy matmuls so the PE stays at
            # K=8/8 through the ~3.5us chain latency (else HAM re-throttles
            # and every post-chain matmul runs at half clock).
            last_chain = pending_norm.pop(0)
            assert not pending_norm

            def tail_mm(ps, mt, n2, kk):
                nc.tensor.matmul(
                    ps,
                    lhsT=ycat_sb[kk][:, mt * 128:(mt + 1) * 128],
                    rhs=wproj_sb[:, kk, n2 * 512:(n2 + 1) * 512],
                    start=(kk == 0), stop=(kk == CL // 128 - 1))

            def tail_out(ps, mt, n2, on_scalar):
                osb = work_p.tile([128, 512], f16, tag="osb", bufs=4,
                                  name="osb")
                if on_scalar:
                    nc.scalar.copy(osb[:], ps)
                else:
                    nc.vector.tensor_copy(osb[:], ps)
                ring = nc.sync if on_scalar else nc.gpsimd
                ring.dma_start(out=out[mt * 128:(mt + 1) * 128,
                                       n2 * 512:(n2 + 1) * 512], in_=osb[:])

            spsA = ps_p.tile([128, JG * 512], f32, tag="sps", name="sps")
            mpsA = ps_p.tile([128, 512], f32, tag="mmps", name="proj_ps")
            mpsB = ps_p.tile([128, 512], f32, tag="mmps", name="proj_ps")
            pre = [(spsA[:, 0:512], 12, 0), (spsA[:, 512:1024], 12, 1),
                   (mpsA[:], 13, 0), (mpsB[:], 13, 1)]
            for ps, mt, n2 in pre:
                tail_mm(ps, mt, n2, 0)
            spsB = ps_p.tile([128, JG * 512], f32, tag="sps", name="sps")
            last_chain()
            for i in range(14):   # warm-keepers; results never read
                sl = (i % 2) * 512
                nc.tensor.matmul(spsB[:, sl:sl + 512], lhsT=wrm[:],
                                 rhs=qk_sb[0][:, 0:512], start=True, stop=True)
            for s, (ps, mt, n2) in enumerate(pre):
                tail_mm(ps, mt, n2, 1)
                tail_out(ps, mt, n2, on_scalar=(s % 2 == 0))
            for n2 in range(2):   # mt 14 on the warm-keeper tile
                for kk in range(CL // 128):
                    tail_mm(spsB[:, n2 * 512:(n2 + 1) * 512], 14, n2, kk)
                tail_out(spsB[:, n2 * 512:(n2 + 1) * 512], 14, n2,
                         on_scalar=(n2 == 0))
            for n2 in range(2):   # mt 15 on recycled mm-pool tiles
                mps = ps_p.tile([128, 512], f32, tag="mmps", name="proj_ps")
                for kk in range(CL // 128):
                    tail_mm(mps[:], 15, n2, kk)
                tail_out(mps[:], 15, n2, on_scalar=(n2 == 0))

    nc.compile()
    return nc


def _shard_inputs(x, w_qkv, w_proj, pe, t_len=T):
    x = np.asarray(x, dtype=np.float32)
    pe = np.asarray(pe, dtype=np.float32)
    xp = (x + pe[None, :t_len, :]).astype(np.float16)   # fold the constant PE
    w_qkv = np.asarray(w_qkv, dtype=np.float32).astype(np.float16)
    w_proj = np.asarray(w_proj, dtype=np.float32).astype(np.float16)

    def chunk_major(a_t):      # [C, t] -> [nt, 128, KC, 512]
        return np.ascontiguousarray(
            a_t.reshape(KC, 128, t_len // 512, 512).transpose(2, 1, 0, 3))

    def part_tiled(w_t):       # [C_in, M] -> [128, C_in//128, M]
        return np.ascontiguousarray(
            w_t.reshape(-1, 128, w_t.shape[1]).transpose(1, 0, 2))

    x_ts = [chunk_major(xp[b, :t_len].T) for b in range(x.shape[0])]
    in_maps = []
    for core in range(NCORES):
        b, g = core // GROUPS, core % GROUPS
        rows_q = w_qkv[g * CL:(g + 1) * CL]
        rows_k = w_qkv[C + g * CL:C + (g + 1) * CL]
        rows_v = w_qkv[2 * C + g * CL:2 * C + (g + 1) * CL]
        in_maps.append({
            "x_t": x_ts[b],
            "w_qk_t": part_tiled(np.concatenate([rows_q, rows_k], axis=0).T.copy()),
            "w_v_t": part_tiled(rows_v.T.copy()),
            "w_proj_t": part_tiled(w_proj[:, g * CL:(g + 1) * CL].T.copy()),
        })
    return in_maps


_RUN_KWARGS = {}       # test-harness hook (e.g. trace=True); empty when graded
_LAST_RESULT = None


def kernel(x, w_qkv, w_proj, pe):
    global _LAST_RESULT
    from concourse import bass_utils

    if T not in _PROG_CACHE:
        _PROG_CACHE[T] = _build_program(T)
    nc = _PROG_CACHE[T]

    in_maps = _shard_inputs(x, w_qkv, w_proj, pe)
    res = bass_utils.run_bass_kernel_spmd(nc, in_maps, core_ids=list(range(NCORES)),
                                          **_RUN_KWARGS)
    _LAST_RESULT = res

    out = np.zeros((B, T, C), dtype=np.float32)
    for core in range(NCORES):
        out[core // GROUPS] += res.results[core]["out"].astype(np.float32)
    return out
